# revision 3
# baseline (speedup 1.0000x reference)
"""A3TGCN RecurrentGCN kernel for 8 TRN2 NeuronCores.

Strategy (dest-sharded, two NEFFs, zero collectives):
  - Append self-loops (c, c, 1.0); shard edges by destination across 8 cores.
  - Per core: sort destinations by in-degree desc, stripe across 128
    partitions (rank r -> partition r%128, stripe r//128); pad each stripe
    to a common (across cores) degree dpad[j]; merge equal-dpad stripes
    into rectangle groups.
  - NEFF-1: segmented-reduce of the padded edge-weight table -> deg,
    1/sqrt -> dinv (all rectangles).
  - Host (pure index routing): route dinv to per-edge dinv[row[e]] slots;
    gather raw x rows into the padded bf16 message table M0.
  - NEFF-2: s = w * dinv_row (f32->bf16), product M0 *= s (broadcast over
    16 features), per-group segmented reduce -> agg, postscale by
    dinv[col], PE transpose, gate matmuls (folded weights computed on
    device), sigmoid/tanh, H=(1-Z)*Ht, relu, classifier, batched softmax.
All floating-point arithmetic happens on device; the host only permutes /
duplicates arrays (sorting, padding, gathering rows by index).
"""
import numpy as np
import ml_dtypes

import concourse.bass as bass
import concourse.bacc as bacc
import concourse.tile as tile
import concourse.mybir as mybir
from concourse import bass_utils
from concourse.ap import AP

F32 = mybir.dt.float32
BF16 = mybir.dt.bfloat16
AF = mybir.ActivationFunctionType
ALU = mybir.AluOpType
BF16_NP = ml_dtypes.bfloat16

N_CORES = 8


# ----------------------------------------------------------------------
# host-side index-space preprocessing
# ----------------------------------------------------------------------

def host_prep(N, NF, edge_index, edge_weight):
    PER = (N + N_CORES - 1) // N_CORES
    NRANK = ((PER + 127) // 128) * 128
    NJ = NRANK // 128

    row = np.asarray(edge_index[0])
    col = np.asarray(edge_index[1])
    w = np.asarray(edge_weight)
    loops = np.arange(N, dtype=row.dtype)
    rows = np.concatenate([row, loops])
    cols = np.concatenate([col, loops])
    ws = np.concatenate([w, np.ones((N,), w.dtype)])

    cores = []
    deg_seq = np.zeros((N_CORES, NRANK), np.int64)
    for k in range(N_CORES):
        lo, hi = k * PER, min((k + 1) * PER, N)
        sel = (cols >= lo) & (cols < hi)
        r_k = rows[sel]
        c_k = (cols[sel] - lo).astype(np.int64)
        w_k = ws[sel]
        cnt = np.bincount(c_k, minlength=PER)
        order = np.argsort(-cnt, kind="stable")
        rank_of = np.empty(PER, np.int64)
        rank_of[order] = np.arange(PER)
        deg_sorted = np.zeros(NRANK, np.int64)
        deg_sorted[:PER] = cnt[order]
        deg_seq[k] = deg_sorted
        cores.append(dict(lo=lo, hi=hi, r=r_k, c=c_k, w=w_k,
                          order=order, rank_of=rank_of))

    dmax = deg_seq.reshape(N_CORES, NJ, 128).max(axis=(0, 2))
    dpad = np.maximum(((dmax + 3) // 4) * 4, 4).astype(np.int64)
    for j in range(NJ - 2, -1, -1):
        dpad[j] = max(dpad[j], dpad[j + 1])
    base = np.concatenate([[0], np.cumsum(dpad)])
    S = int(base[-1])

    groups = []
    j = 0
    while j < NJ:
        j2 = j
        while j2 < NJ and dpad[j2] == dpad[j]:
            j2 += 1
        groups.append((j, j2 - j, int(dpad[j])))
        j = j2

    meta = dict(N=N, NF=NF, PER=PER, NRANK=NRANK, NJ=NJ, S=S,
                dpad=dpad, base=base, groups=groups)

    for k, ck in enumerate(cores):
        rank_e = ck["rank_of"][ck["c"]]
        eord = np.argsort(rank_e, kind="stable")
        rank_s = rank_e[eord]
        r_s = ck["r"][eord]
        w_s = ck["w"][eord]
        t = np.arange(len(rank_s)) - np.searchsorted(rank_s, rank_s, side="left")
        ck.update(p_e=rank_s % 128, j_e=rank_s // 128, t_e=t, r_s=r_s, w_s=w_s)
    return meta, cores


def host_tables(meta, cores, x):
    S, base, dpad = meta["S"], meta["base"], meta["dpad"]
    NF = meta["NF"]
    xbf = x.astype(BF16_NP)
    out = []
    for ck in cores:
        p_e, j_e, t_e, r_s, w_s = ck["p_e"], ck["j_e"], ck["t_e"], ck["r_s"], ck["w_s"]
        M0 = np.zeros((128, 16 * S), BF16_NP)
        w_pad = np.zeros((128, S), BF16_NP)
        d_e = dpad[j_e]
        fbase = 16 * base[j_e] + t_e
        for f in range(NF):
            M0[p_e, fbase + f * d_e] = xbf[r_s, f]
        w_pad[p_e, base[j_e] + t_e] = w_s.astype(BF16_NP)
        ndum = meta["NRANK"] - meta["PER"]
        if ndum > 0:
            dum = np.arange(meta["PER"], meta["NRANK"])
            w_pad[dum % 128, base[dum // 128]] = np.maximum(
                w_pad[dum % 128, base[dum // 128]], 1.0)
        out.append(dict(M0=M0, w_pad=w_pad))
    return out


def host_dinvrow(meta, cores, dinv_tiles):
    N, PER = meta["N"], meta["PER"]
    base = meta["base"]
    dinv_node = np.zeros(N, np.float32)
    for k, ck in enumerate(cores):
        dv = np.asarray(dinv_tiles[k])
        ranks = ck["rank_of"]
        dinv_node[ck["lo"]:ck["hi"]] = dv[ranks % 128, ranks // 128]
    outs = []
    for ck in cores:
        dr = np.zeros((128, meta["S"]), BF16_NP)
        dr[ck["p_e"], base[ck["j_e"]] + ck["t_e"]] = dinv_node[ck["r_s"]].astype(BF16_NP)
        outs.append(dr)
    return outs, dinv_node


# ----------------------------------------------------------------------
# device builders
# ----------------------------------------------------------------------

def ap3(t, off, dims):
    a = t[:]
    return AP(a.tensor, a.offset + off, [list(a.ap[0])] + [list(d) for d in dims])


def build_neff1(nc, meta):
    S, NJ, groups = meta["S"], meta["NJ"], meta["groups"]
    w_in = nc.dram_tensor("w_pad", [128, S], BF16, kind="ExternalInput")
    dinv_out = nc.dram_tensor("dinv", [128, NJ], F32, kind="ExternalOutput")
    with tile.TileContext(nc) as tc:
        with tc.tile_pool(name="p", bufs=1) as pool:
            wt = pool.tile([128, S], BF16)
            deg = pool.tile([128, NJ], F32)
            sq = pool.tile([128, NJ], F32)
            dv = pool.tile([128, NJ], F32)
            nh = (len(groups) + 1) // 2
            for grs in (groups[:nh], groups[nh:]):
                if not grs:
                    continue
                b0 = int(meta["base"][grs[0][0]])
                b1 = int(meta["base"][grs[-1][0] + grs[-1][1]])
                nc.sync.dma_start(wt[:, b0:b1], w_in.ap()[:, b0:b1])
                for (j0, nj, d) in grs:
                    src = ap3(wt, int(meta["base"][j0]), [(d, nj), (1, d)])
                    dst = ap3(deg, j0, [(1, nj)])
                    nc.vector.tensor_reduce(dst, src, mybir.AxisListType.X, ALU.add)
            nc.scalar.activation(sq[:], deg[:], AF.Sqrt)
            nc.vector.reciprocal(dv[:], sq[:])
            nc.sync.dma_start(dinv_out.ap(), dv[:])
    nc.compile()
    return nc


def build_neff2(nc, meta, SF, NCLS):
    S, NJ, groups, base = meta["S"], meta["NJ"], meta["groups"], meta["base"]

    m0_in = nc.dram_tensor("M0", [128, 16 * S], BF16, kind="ExternalInput")
    w_in = nc.dram_tensor("w_pad", [128, S], BF16, kind="ExternalInput")
    dr_in = nc.dram_tensor("dinvrow", [128, S], BF16, kind="ExternalInput")
    dinv_in = nc.dram_tensor("dinv", [128, NJ], F32, kind="ExternalInput")
    wzT_in = nc.dram_tensor("WzT", [SF, 16], F32, kind="ExternalInput")
    whT_in = nc.dram_tensor("WhT", [SF, 16], F32, kind="ExternalInput")
    lz_in = nc.dram_tensor("Lzt", [SF, SF], F32, kind="ExternalInput")
    lh_in = nc.dram_tensor("Lht", [SF, SF], F32, kind="ExternalInput")
    bz_in = nc.dram_tensor("bz", [SF, 1], F32, kind="ExternalInput")
    bh_in = nc.dram_tensor("bh", [SF, 1], F32, kind="ExternalInput")
    lzb_in = nc.dram_tensor("Lz_b", [SF, 1], F32, kind="ExternalInput")
    lhb_in = nc.dram_tensor("Lh_b", [SF, 1], F32, kind="ExternalInput")
    ow_in = nc.dram_tensor("out_W", [SF, NCLS], F32, kind="ExternalInput")
    ob_in = nc.dram_tensor("out_b_t", [128, NCLS], F32, kind="ExternalInput")
    ident_in = nc.dram_tensor("ident", [128, 128], BF16, kind="ExternalInput")
    sel32_in = nc.dram_tensor("sel32", [SF, 96], F32, kind="ExternalInput")
    probs_out = nc.dram_tensor("probs", [128, NJ * NCLS], F32, kind="ExternalOutput")

    with tile.TileContext(nc) as tc:
        with (
            tc.tile_pool(name="big", bufs=2) as big,
            tc.tile_pool(name="sm", bufs=1) as sm,
            tc.tile_pool(name="gp", bufs=2) as gp,
            tc.tile_pool(name="gps", bufs=2, space="PSUM") as gps,
        ):
            wt = sm.tile([128, S], BF16, tag="w")
            drt = sm.tile([128, S], BF16, tag="dr")
            st = sm.tile([128, S], BF16, tag="s")
            dvt = sm.tile([128, NJ], F32, tag="dv")
            nc.sync.dma_start(dvt[:], dinv_in.ap())

            ident = sm.tile([128, 128], BF16, tag="id")
            nc.sync.dma_start(ident[:], ident_in.ap())
            sel32 = sm.tile([SF, 96], F32, tag="sel32")
            nc.sync.dma_start(sel32[:], sel32_in.ap())

            lz = sm.tile([SF, SF], F32, tag="lz")
            lh = sm.tile([SF, SF], F32, tag="lh")
            wzT = sm.tile([SF, 16], F32, tag="wzT")
            whT = sm.tile([SF, 16], F32, tag="whT")
            bz = sm.tile([SF, 1], F32, tag="bz")
            bh = sm.tile([SF, 1], F32, tag="bh")
            lzb = sm.tile([SF, 1], F32, tag="lzb")
            lhb = sm.tile([SF, 1], F32, tag="lhb")
            ow = sm.tile([SF, NCLS], F32, tag="ow")
            obt = sm.tile([128, NCLS], F32, tag="obt")
            for t_, i_ in ((lz, lz_in), (lh, lh_in), (wzT, wzT_in), (whT, whT_in),
                           (bz, bz_in), (bh, bh_in), (lzb, lzb_in), (lhb, lhb_in),
                           (ow, ow_in), (obt, ob_in)):
                nc.sync.dma_start(t_[:], i_.ap())

            # fold gate weights on device
            weffz_p = gps.tile([16, 512], F32, tag="zp")
            weffh_p = gps.tile([16, 512], F32, tag="hp")
            bz_p = gps.tile([SF, 512], F32, tag="aggTp")
            bh_p = gps.tile([128, 512], F32, tag="lgp")
            weffz = sm.tile([16, SF], BF16, tag="weffz")
            weffh = sm.tile([16, SF], BF16, tag="weffh")
            bzv = sm.tile([SF, 1], F32, tag="bzv")
            bhv = sm.tile([SF, 1], F32, tag="bhv")
            nc.tensor.matmul(weffz_p[:, :SF], wzT[:], lz[:])
            nc.tensor.matmul(weffh_p[:, :SF], whT[:], lh[:])
            nc.vector.tensor_copy(weffz[:], weffz_p[:, :SF])
            nc.vector.tensor_copy(weffh[:], weffh_p[:, :SF])
            nc.tensor.matmul(bz_p[:SF, :1], lz[:], bz[:])
            nc.tensor.matmul(bh_p[:SF, :1], lh[:], bh[:])
            nc.vector.tensor_add(bzv[:], bz_p[:SF, :1], lzb[:])
            nc.vector.tensor_add(bhv[:], bh_p[:SF, :1], lhb[:])
            # replicate [SF,1] biases to [128,1] via tiled-identity matmul
            bzv4_p = gps.tile([128, 512], F32, tag="lgp")
            bhv4_p = gps.tile([128, 512], F32, tag="lgp")
            bzv4 = sm.tile([96, 1], F32, tag="bzv4")
            bhv4 = sm.tile([96, 1], F32, tag="bhv4")
            nc.tensor.matmul(bzv4_p[:96, :1], sel32[:], bzv[:])
            nc.tensor.matmul(bhv4_p[:96, :1], sel32[:], bhv[:])
            nc.vector.tensor_copy(bzv4[:], bzv4_p[:96, :1])
            nc.vector.tensor_copy(bhv4[:], bhv4_p[:96, :1])
            owb = sm.tile([96, NCLS], BF16, tag="owb")
            for c in range(3):
                nc.gpsimd.dma_start(owb[32 * c:32 * (c + 1), :], ow_in.ap())

            agg = sm.tile([128, NJ * 16], F32, tag="agg")
            aggb = sm.tile([128, NJ * 16], BF16, tag="aggb")
            aggT = sm.tile([16, NJ * 128], BF16, tag="aggT")
            NODES = NJ * 128
            HB = ((NODES + 1535) // 1536 + 8) * 512
            hhb = sm.tile([96, HB], BF16, tag="hhb")
            probs_sb = sm.tile([128, NJ * NCLS], F32, tag="probs")

            # split groups into ~4 pieces by table volume
            total_elems = int(16 * base[NJ])
            tgt = (total_elems + 9) // 10
            halves = []
            cur, acc = [], 0
            maxel = 0
            for g in groups:
                cur.append(g)
                acc += 16 * g[2] * g[1]
                if acc >= tgt:
                    j0h, j1h = cur[0][0], cur[-1][0] + cur[-1][1]
                    maxel = max(maxel, int(16 * (base[j1h] - base[j0h])))
                    halves.append((len(halves), cur, j0h, j1h))
                    cur, acc = [], 0
            if cur:
                j0h, j1h = cur[0][0], cur[-1][0] + cur[-1][1]
                maxel = max(maxel, int(16 * (base[j1h] - base[j0h])))
                halves.append((len(halves), cur, j0h, j1h))

            sc_ctr = [0]
            for (hidx, grs, j0h, j1h) in halves:
                e0, e1 = int(16 * base[j0h]), int(16 * base[j1h])
                s0e, s1e = int(base[j0h]), int(base[j1h])
                nc.sync.dma_start(wt[:, s0e:s1e], w_in.ap()[:, s0e:s1e])
                nc.sync.dma_start(drt[:, s0e:s1e], dr_in.ap()[:, s0e:s1e])
                nc.gpsimd.tensor_tensor(st[:, s0e:s1e], wt[:, s0e:s1e],
                                        drt[:, s0e:s1e], ALU.mult)
                mt = big.tile([128, maxel], BF16, tag="m")
                nc.sync.dma_start(mt[:, :e1 - e0], m0_in.ap()[:, e0:e1])
                for (j0, nj, d) in grs:
                    off = int(16 * base[j0]) - e0
                    msrc = ap3(mt, off, [(16 * d, nj), (d, 16), (1, d)])
                    ssrc = ap3(st, int(base[j0]), [(d, nj), (0, 16), (1, d)])
                    nc.vector.tensor_mul(msrc, msrc, ssrc)
                    # 2-level pairwise tree then reduce the d/4 tail
                    dd = d
                    while dd % 2 == 0 and dd > 2:
                        h = dd // 2
                        a0 = ap3(mt, off, [(16 * d, nj), (d, 16), (1, h)])
                        a1 = ap3(mt, off + h, [(16 * d, nj), (d, 16), (1, h)])
                        nc.vector.tensor_add(a0, a0, a1)
                        dd = h
                    rsrc = ap3(mt, off, [(16 * d, nj), (d, 16), (1, dd)])
                    rdst = ap3(agg, j0 * 16, [(16, nj), (1, 16)])
                    nc.vector.tensor_reduce(rdst, rsrc, mybir.AxisListType.X, ALU.add)
                # postscale by dinv[col] -> bf16
                njh = j1h - j0h
                agg3 = ap3(agg, j0h * 16, [(16, njh), (1, 16)])
                aggb3 = ap3(aggb, j0h * 16, [(16, njh), (1, 16)])
                dv_b = ap3(dvt, j0h, [(1, njh), (0, 16)])
                nc.vector.tensor_mul(aggb3, agg3, dv_b)

                # transposes for this piece, batched 8 stripes per PSUM tile
                for jb in range(j0h, j1h, 8):
                    njb = min(8, j1h - jb)
                    tp_p = gps.tile([16, 1024], BF16, tag="aggTp")
                    for jj in range(njb):
                        nc.tensor.transpose(
                            tp_p[:, jj * 128:(jj + 1) * 128],
                            ap3(aggb, (jb + jj) * 16, [(1, 16)]),
                            ident[:],
                        )
                    nc.scalar.activation(
                        aggT[:, jb * 128:(jb + njb) * 128], tp_p[:, :njb * 128],
                        AF.Copy)

                # gate phase: 3-stacked super-chunks of 1536 nodes
                n0h, n1h = j0h * 128, j1h * 128
                sc_tab = {}
                for sc0 in range(n0h, n1h, 1536):
                    scn = min(1536, n1h - sc0)
                    nsub = (scn + 511) // 512
                    wid = min(512, scn)
                    hb0 = 512 * sc_ctr[0]
                    sc_tab[sc0] = hb0
                    sc_ctr[0] += 1
                    zpre_p = gps.tile([128, 512], F32, tag="zp")
                    hpre_p = gps.tile([128, 512], F32, tag="hp")
                    for c in range(nsub):
                        cn = min(512, scn - c * 512)
                        nc.tensor.matmul(zpre_p[32 * c:32 * c + SF, :cn], weffz[:],
                                         aggT[:, sc0 + c * 512: sc0 + c * 512 + cn])
                        nc.tensor.matmul(hpre_p[32 * c:32 * c + SF, :cn], weffh[:],
                                         aggT[:, sc0 + c * 512: sc0 + c * 512 + cn])
                    zt = gp.tile([96, 512], F32, tag="zt")
                    ht = gp.tile([96, 512], F32, tag="ht")
                    nc.scalar.activation(zt[:, :wid], zpre_p[:96, :wid], AF.Sigmoid, bias=bzv4[:])
                    nc.scalar.activation(ht[:, :wid], hpre_p[:96, :wid], AF.Tanh, bias=bhv4[:])
                    hh = gp.tile([96, 512], F32, tag="hh")
                    nc.vector.scalar_tensor_tensor(
                        hh[:, :wid], zt[:, :wid], 1.0, ht[:, :wid],
                        ALU.subtract, ALU.mult)
                    nc.scalar.activation(hhb[:, hb0:hb0 + wid], hh[:, :wid],
                                         AF.Relu, scale=-1.0)

                # logits per 128-node tile; bias add on idle gpsimd
                for j in range(j0h, j1h):
                    n = j * 128
                    rel = n - n0h
                    sc0 = n0h + (rel // 1536) * 1536
                    c = (n - sc0) // 512
                    coff = sc_tab[sc0] + (n - sc0 - c * 512)
                    lg_p = gps.tile([128, 512], F32, tag="lgp")
                    nc.tensor.matmul(lg_p[:, :NCLS],
                                     hhb[32 * c:32 * c + SF, coff:coff + 128],
                                     owb[32 * c:32 * c + SF, :])
                    nc.vector.tensor_add(
                        probs_sb[:, j * NCLS:(j + 1) * NCLS],
                        lg_p[:, :NCLS], obt[:])

            # batched softmax over classes
            lg3 = ap3(probs_sb, 0, [(NCLS, NJ), (1, NCLS)])
            nc.scalar.activation(probs_sb[:], probs_sb[:], AF.Exp)
            sme = sm.tile([128, NJ], F32, tag="sme")
            nc.vector.tensor_reduce(ap3(sme, 0, [(1, NJ)]), lg3,
                                    mybir.AxisListType.X, ALU.add)
            rcp = sm.tile([128, NJ], F32, tag="rcp")
            nc.vector.reciprocal(rcp[:], sme[:])
            rcp_b = ap3(rcp, 0, [(1, NJ), (0, NCLS)])
            nc.vector.tensor_mul(lg3, lg3, rcp_b)
            nc.sync.dma_start(probs_out.ap(), probs_sb[:])
    nc.compile()
    return nc


# ----------------------------------------------------------------------
# orchestration
# ----------------------------------------------------------------------

def gnn_kernel(x, edge_index, edge_weight, Wz, bz, Wr, br, Wh, bh,
               Lz_W, Lz_b, Lr_W, Lr_b, Lh_W, Lh_b, out_W, out_b, attention,
               trace=False):
    N, NF = x.shape
    SF = Wz.shape[1]
    NCLS = out_W.shape[1]
    x = np.asarray(x, np.float32)
    meta, cores = host_prep(N, NF, np.asarray(edge_index),
                            np.asarray(edge_weight, np.float32))
    tabs = host_tables(meta, cores, x)

    nc1 = bacc.Bacc("TRN2", target_bir_lowering=False, debug=False,
                    num_devices=N_CORES)
    build_neff1(nc1, meta)
    in1 = [{"w_pad": tabs[k]["w_pad"]} for k in range(N_CORES)]
    r1 = bass_utils.run_bass_kernel_spmd(nc1, in1, core_ids=list(range(N_CORES)),
                                         trace=trace)
    dinv_tiles = [r1.results[k]["dinv"] for k in range(N_CORES)]

    drows, dinv_node = host_dinvrow(meta, cores, dinv_tiles)

    nc2 = bacc.Bacc("TRN2", target_bir_lowering=False, debug=False,
                    num_devices=N_CORES)
    build_neff2(nc2, meta, SF, NCLS)
    common = {
        "WzT": np.ascontiguousarray(np.asarray(Wz, np.float32).T),
        "WhT": np.ascontiguousarray(np.asarray(Wh, np.float32).T),
        "Lzt": np.ascontiguousarray(np.asarray(Lz_W, np.float32)[:SF]),
        "Lht": np.ascontiguousarray(np.asarray(Lh_W, np.float32)[:SF]),
        "bz": np.asarray(bz, np.float32).reshape(SF, 1),
        "bh": np.asarray(bh, np.float32).reshape(SF, 1),
        "Lz_b": np.asarray(Lz_b, np.float32).reshape(SF, 1),
        "Lh_b": np.asarray(Lh_b, np.float32).reshape(SF, 1),
        "out_W": np.ascontiguousarray(np.asarray(out_W, np.float32)),
        "out_b_t": np.tile(np.asarray(out_b, np.float32)[None, :], (128, 1)),
        "ident": np.eye(128, dtype=BF16_NP),
        "sel32": np.tile(np.eye(32, dtype=np.float32), (1, 3)),
    }
    in2 = [dict(common, M0=tabs[k]["M0"], w_pad=tabs[k]["w_pad"],
                dinvrow=drows[k], dinv=np.asarray(dinv_tiles[k]))
           for k in range(N_CORES)]
    r2 = bass_utils.run_bass_kernel_spmd(nc2, in2, core_ids=list(range(N_CORES)),
                                         trace=trace)
    global LAST_RESULTS
    LAST_RESULTS = (r1, r2)

    out = np.zeros((N, NCLS), np.float32)
    for k, ck in enumerate(cores):
        pr = np.asarray(r2.results[k]["probs"]).reshape(128, meta["NJ"], NCLS)
        ranks = ck["rank_of"]
        out[ck["lo"]:ck["hi"]] = pr[ranks % 128, ranks // 128]
    return out, (r1.exec_time_ns, r2.exec_time_ns)


# ----------------------------------------------------------------------
# harness entry point
# ----------------------------------------------------------------------

LAST_EXEC_NS = None
LAST_RESULTS = None


def kernel(**inputs):
    """Full inputs in, full output out. Shards across 8 NeuronCores
    internally (two SPMD NEFFs with host-side index routing in between)."""
    global LAST_EXEC_NS
    import os
    trace = bool(os.environ.get("GNN_TRACE"))
    out, times = gnn_kernel(**inputs, trace=trace)
    LAST_EXEC_NS = times
    return out

